# revision 11
# baseline (speedup 1.0000x reference)
"""Causal self-attention kernel for Trainium2, 8-core data parallel.

Per-core program: one batch element b of x [8, 1024, 768].
  - x, W_attn, W_proj transposed into c-on-partitions layouts via
    cast-to-bf16 + DRAM round-trip DMA transpose.
  - qkT (Q features) / KTpad (K features, zero-padded to 128 rows for FWL)
    / V [t, j]: weight-reuse-ordered GEMMs.
  - per head: S^T = K^T.T @ Q^T (k on partitions, q free), P^T = exp(S^T/8)
    with causal tri-mask on diagonal blocks; y^T_h = V^T-slices @ P^T with a
    concurrent col-tiled ones-matmul accumulating softmax denominators;
    normalized via PE-broadcast reciprocal.
  - out = y @ Wp^T + b_p (t on partitions).
All matmuls bf16 inputs / fp32 PSUM accumulation; softmax in fp32.
"""
import sys
import contextlib
from contextlib import ExitStack

sys.path.insert(0, "/opt/trn_rl_repo")

import numpy as np

import concourse.bass as bass
import concourse.bacc as bacc
import concourse.mybir as mybir
import concourse.tile as tile
from concourse.masks import make_upper_triangular

F32 = mybir.dt.float32
BF16 = mybir.dt.bfloat16

P = 128
T = 1024
C = 768
H = 12
HS = 64
CT = C // P     # 6 c-tiles
TT = T // P     # 8 t-tiles
KT = T // P     # 8 k-tiles per head
N_CORES = 8


def qk_chunks(kt):
    """[(q0, w)] matmul chunks for k-tile kt: q >= kt*128, each split in two
    for stationary-weight reuse."""
    qs = kt * P
    if kt < 4:
        return [(qs, 512 - qs), (512, 512)]
    return [(qs, T - qs)]


def pv_chunks(kt, b):
    """[(off, w)] chunks within q-block b for k-tile kt."""
    off = max(0, kt * P - b * 512)
    return [(off, 512 - off)]


def emit_consts(nc, tc, const, ba_d, bp_d):
    from concourse.masks import make_identity
    ident = const.tile([P, P], F32, tag="ident")
    make_identity(nc, ident)
    tri = const.tile([P, P], BF16, tag="tri")
    make_upper_triangular(nc, tri, val=1.0, diag=True)
    ones_bf = const.tile([P, P], BF16, tag="ones")
    nc.gpsimd.memset(ones_bf[:], 1.0)
    bias_qk = const.tile([P, 12], F32, tag="bqk")
    nc.sync.dma_start(bias_qk[:], ba_d[0 : 2 * C].rearrange("(o p) -> p o", p=P))
    Bv = const.tile([P, C], F32, tag="Bv")
    nc.sync.dma_start(
        Bv[:],
        ba_d[2 * C : 3 * C].rearrange("(a j) -> a j", a=1).to_broadcast([P, C]),
    )
    Bp = const.tile([P, C], F32, tag="Bp")
    nc.sync.dma_start(
        Bp[:], bp_d.rearrange("(a j) -> a j", a=1).to_broadcast([P, C])
    )
    return dict(ident=ident, tri=tri, ones_bf=ones_bf, bias_qk=bias_qk, Bv=Bv, Bp=Bp)


def emit_body(nc, tc, pools, cst, dram, x_d, wa_d, wp_d, y_d):
    const, persist, nat, work, ptp, ps512, ps384 = pools
    tri, ones_bf = cst["tri"], cst["ones_bf"]
    bias_qk, Bv, Bp = cst["bias_qk"], cst["Bv"], cst["Bp"]

    xT = persist.tile([P, CT, T], BF16, tag="xT")
    WT = persist.tile([P, CT, 3 * C], BF16, tag="WT")
    WpT = persist.tile([P, CT, C], BF16, tag="WpT")
    qT = persist.tile([P, CT, T], BF16, tag="qT")         # Q features, pair layout
    KTp = persist.tile([P, 12, T], BF16, tag="KTp")       # per-head, zero-padded
    V = persist.tile([P, TT, 12, 65], BF16, tag="Vaug")
    yT = persist.tile([P, CT, T], BF16, tag="yT")

    # ---- phase 0: cast bf16 + DRAM round-trip transpose.
    # Transpose DMAs ride ACT's HWDGE queue, scratch stores ride DVE's, other
    # DMAs SP's -- keeps xbar-mode transitions off the main copy queues.
    def transpose_in(dram_src, n_row_tiles, scratch, dst, dst_cols):
        src = dram_src.rearrange("(rt p) c -> rt p c", p=P)
        sc = scratch.rearrange("(rt p) c -> rt p c", p=P)
        for rt in range(n_row_tiles):
            natt = nat.tile([P, C], F32, tag="nat")
            nc.sync.dma_start(natt[:], src[rt])
            natb = nat.tile([P, C], BF16, tag="natb")
            if rt % 2 == 0:
                nc.vector.tensor_copy(natb[:], natt[:])
            else:
                nc.scalar.activation(
                    natb[:], natt[:], mybir.ActivationFunctionType.Copy
                )
            nc.gpsimd.dma_start(sc[rt], natb[:])
        for ci in range(CT):
            nc.scalar.dma_start(
                dst[:, ci, :dst_cols],
                scratch[:, ci * P : (ci + 1) * P],
                transpose=True,
            )

    wa_bf = dram.tile([3 * C, C], BF16, tag="wa_bf")
    xs_bf = dram.tile([T, C], BF16, tag="xs_bf")
    wp_bf = dram.tile([C, C], BF16, tag="wp_bf")
    transpose_in(wa_d, 18, wa_bf, WT, 3 * C)
    transpose_in(x_d, TT, xs_bf, xT, T)
    transpose_in(wp_d, CT, wp_bf, WpT, C)

    # ---- phase 1: Q -> qT (pair layout), K -> KTp (padded per head), V [t, j]
    for jt in range(12):
        pss = [ps512.tile([P, 512], F32, tag="ps512", name=f"qkps{i}") for i in range(2)]
        for ct in range(CT):
            for tb in range(2):
                nc.tensor.matmul(
                    pss[tb][:],
                    WT[:, ct, jt * P : (jt + 1) * P],
                    xT[:, ct, tb * 512 : (tb + 1) * 512],
                    start=(ct == 0),
                    stop=(ct == CT - 1),
                )
        for tb in range(2):
            sl = slice(tb * 512, (tb + 1) * 512)
            if jt < 6:
                nc.vector.tensor_scalar_add(
                    qT[:, jt, sl], pss[tb][:], bias_qk[:, jt : jt + 1]
                )
            else:
                j = jt - 6
                nc.vector.tensor_scalar_add(
                    KTp[0:64, 2 * j, sl], pss[tb][0:64, :],
                    bias_qk[0:64, jt : jt + 1],
                )
                nc.vector.tensor_scalar_add(
                    KTp[64:128, 2 * j + 1, sl], pss[tb][64:128, :],
                    bias_qk[64:128, jt : jt + 1],
                )
    for tt in range(TT):
        pss = [ps384.tile([P, 384], F32, tag="ps384", name=f"vps{i}") for i in range(2)]
        for ct in range(CT):
            for jb in range(2):
                nc.tensor.matmul(
                    pss[jb][:],
                    xT[:, ct, tt * P : (tt + 1) * P],
                    WT[:, ct, 2 * C + jb * 384 : 2 * C + (jb + 1) * 384],
                    start=(ct == 0),
                    stop=(ct == CT - 1),
                )
        for jb in range(2):
            sl = slice(jb * 384, (jb + 1) * 384)
            nc.vector.tensor_add(
                V[:, tt, 6 * jb : 6 * jb + 6, 0:64],
                pss[jb][:].rearrange("p (h d) -> p h d", d=64),
                Bv[:, sl].rearrange("p (h d) -> p h d", d=64),
            )

    # ---- phase 2: per-head attention
    for h in range(H):
        hb = 64 * (h % 2)       # partition base of this head's rows
        vb = hb                 # psum col base for PV output
        db = 64 - hb            # psum col base for denominator
        qj = h // 2

        PT = ptp.tile([P, KT, T], BF16, tag="PT")
        for kt in range(KT):
            qs = kt * P
            for (q0, w) in qk_chunks(kt):
                sps = ps512.tile([P, 512], F32, tag="ps512")
                nc.tensor.matmul(
                    sps[:, :w],
                    KTp[:, h, kt * P : (kt + 1) * P],
                    qT[:, qj, q0 : q0 + w],
                    start=True,
                    stop=True,
                )
                nc.scalar.activation(
                    PT[:, kt, q0 : q0 + w],
                    sps[:, :w],
                    mybir.ActivationFunctionType.Exp,
                    scale=0.125,
                )
            nc.gpsimd.affine_select(
                out=PT[:, kt, qs : qs + P],
                in_=PT[:, kt, qs : qs + P],
                compare_op=mybir.AluOpType.is_ge,
                fill=0.0,
                base=0,
                # keep where (-k + q') >= 0, else fill 0
                pattern=[[1, P]],
                channel_multiplier=-1,
            )

        for b in range(2):
            yD = ps512.tile([P, 512], F32, tag="ps512")
            n_kt = 4 if b == 0 else 8
            for kt in range(n_kt):
                for (off, w) in pv_chunks(kt, b):
                    first = kt == 0
                    last = kt == n_kt - 1
                    rhs = PT[:, kt, b * 512 + off : b * 512 + off + w]
                    nc.tensor.matmul(
                        yD[0:65, off : off + w],
                        V[:, kt, h, :],
                        rhs,
                        start=first,
                        stop=last,
                    )
            Dr = work.tile([P, 512], F32, tag="Dr")
            nc.vector.reciprocal(Dr[64:65, :], yD[64:65, :])
            Drb = work.tile([P, 512], BF16, tag="Drb")
            nc.vector.tensor_copy(Drb[64:65, :], Dr[64:65, :])
            rps = ps512.tile([P, 512], F32, tag="ps512")
            nc.tensor.matmul(
                rps[0:64, :],
                ones_bf[64:65, 0:64],
                Drb[64:65, :],
                start=True,
                stop=True,
                tile_position=(64, 0),
            )
            Rh = work.tile([P, 512], F32, tag="Rh")
            nc.scalar.activation(
                Rh[0:64, :],
                rps[0:64, :],
                mybir.ActivationFunctionType.Copy,
            )
            if hb == 0:
                nc.vector.tensor_mul(
                    yT[0:64, h // 2, b * 512 : (b + 1) * 512],
                    yD[0:64, :],
                    Rh[0:64, :],
                )
            else:
                yTt = work.tile([P, 512], BF16, tag="yTt")
                nc.vector.tensor_mul(yTt[0:64, :], yD[0:64, :], Rh[0:64, :])
                nc.sync.dma_start(
                    yT[64:128, h // 2, b * 512 : (b + 1) * 512], yTt[0:64, :]
                )

    # ---- phase 3: out = y @ Wp^T + b_p
    for tt in range(TT):
        osb = work.tile([P, C], F32, tag="osb")
        pss = [ps384.tile([P, 384], F32, tag="ps384", name=f"vps{i}") for i in range(2)]
        for ct in range(CT):
            for jb in range(2):
                nc.tensor.matmul(
                    pss[jb][:],
                    yT[:, ct, tt * P : (tt + 1) * P],
                    WpT[:, ct, jb * 384 : (jb + 1) * 384],
                    start=(ct == 0),
                    stop=(ct == CT - 1),
                )
        for jb in range(2):
            sl = slice(jb * 384, (jb + 1) * 384)
            nc.vector.tensor_add(osb[:, sl], pss[jb][:], Bp[:, sl])
        nc.sync.dma_start(
            y_d.rearrange("(tt p) c -> tt p c", p=P)[tt], osb[:]
        )


def build_program(loop=1):
    nc = bacc.Bacc("TRN2", target_bir_lowering=False, debug=False)
    x_d = nc.dram_tensor("x", [T, C], F32, kind="ExternalInput").ap()
    wa_d = nc.dram_tensor("W_attn", [3 * C, C], F32, kind="ExternalInput").ap()
    ba_d = nc.dram_tensor("b_attn", [3 * C], F32, kind="ExternalInput").ap()
    wp_d = nc.dram_tensor("W_proj", [C, C], F32, kind="ExternalInput").ap()
    bp_d = nc.dram_tensor("b_proj", [C], F32, kind="ExternalInput").ap()
    y_d = nc.dram_tensor("y", [T, C], F32, kind="ExternalOutput").ap()

    with tile.TileContext(nc) as tc, ExitStack() as ctx:
        const = ctx.enter_context(tc.tile_pool(name="const", bufs=1))
        persist = ctx.enter_context(tc.tile_pool(name="persist", bufs=1))
        nat = ctx.enter_context(tc.tile_pool(name="nat", bufs=4))
        work = ctx.enter_context(tc.tile_pool(name="work", bufs=2))
        ptp = ctx.enter_context(tc.tile_pool(name="ptp", bufs=2))
        dram = ctx.enter_context(tc.tile_pool(name="dram", bufs=1, space="DRAM"))
        ps512 = ctx.enter_context(tc.tile_pool(name="ps512", bufs=6, space="PSUM"))
        ps384 = ctx.enter_context(tc.tile_pool(name="ps384", bufs=2, space="PSUM"))
        pools = (const, persist, nat, work, ptp, ps512, ps384)

        cst = emit_consts(nc, tc, const, ba_d, bp_d)
        # zero-fill KTp's complementary halves + V ones columns once
        KTp0 = persist.tile([P, 12, T], BF16, tag="KTp")
        nc.gpsimd.memset(KTp0[:], 0.0)
        V0 = persist.tile([P, TT, 12, 65], BF16, tag="Vaug")
        nc.gpsimd.memset(V0[:, :, :, 64:65], 1.0)
        loop_cm = tc.For_i(0, loop, 1) if loop > 1 else contextlib.nullcontext()
        with loop_cm:
            emit_body(nc, tc, pools, cst, dram, x_d, wa_d, wp_d, y_d)

    nc.compile()
    return nc


_CACHED_NC = None


def kernel(x, W_attn, b_attn, W_proj, b_proj):
    from concourse.bass_utils import run_bass_kernel_spmd

    global _CACHED_NC
    if _CACHED_NC is None:
        _CACHED_NC = build_program(loop=1)
    nc = _CACHED_NC

    B = x.shape[0]
    assert B == N_CORES
    in_maps = [
        {
            "x": np.ascontiguousarray(x[b], dtype=np.float32),
            "W_attn": np.asarray(W_attn, dtype=np.float32),
            "b_attn": np.asarray(b_attn, dtype=np.float32),
            "W_proj": np.asarray(W_proj, dtype=np.float32),
            "b_proj": np.asarray(b_proj, dtype=np.float32),
        }
        for b in range(B)
    ]
    res = run_bass_kernel_spmd(nc, in_maps, list(range(N_CORES)))
    return np.stack([res.results[b]["y"] for b in range(B)], axis=0)


# revision 12
# speedup vs baseline: 1.1033x; 1.1033x over previous
"""Causal self-attention kernel for Trainium2, 8-core data parallel.

Per-core program: one batch element b of x [8, 1024, 768].
  - x, W_attn, W_proj transposed into c-on-partitions layouts via
    cast-to-bf16 + DRAM round-trip DMA transpose.
  - qkT (Q features) / KTpad (K features, zero-padded to 128 rows for FWL)
    / V [t, j]: weight-reuse-ordered GEMMs.
  - per head: S^T = K^T.T @ Q^T (k on partitions, q free), P^T = exp(S^T/8)
    with causal tri-mask on diagonal blocks; y^T_h = V^T-slices @ P^T with a
    concurrent col-tiled ones-matmul accumulating softmax denominators;
    normalized via PE-broadcast reciprocal.
  - out = y @ Wp^T + b_p (t on partitions).
All matmuls bf16 inputs / fp32 PSUM accumulation; softmax in fp32.
"""
import sys
import contextlib
from contextlib import ExitStack

sys.path.insert(0, "/opt/trn_rl_repo")

import numpy as np

import concourse.bass as bass
import concourse.bacc as bacc
import concourse.mybir as mybir
import concourse.tile as tile
from concourse.masks import make_upper_triangular

F32 = mybir.dt.float32
BF16 = mybir.dt.bfloat16

P = 128
T = 1024
C = 768
H = 12
HS = 64
CT = C // P     # 6 c-tiles
TT = T // P     # 8 t-tiles
KT = T // P     # 8 k-tiles per head
N_CORES = 8


def qk_chunks(kt):
    """[(q0, w)] matmul chunks for k-tile kt: q >= kt*128, each split in two
    for stationary-weight reuse."""
    qs = kt * P
    if kt < 4:
        return [(qs, 512 - qs), (512, 512)]
    return [(qs, T - qs)]


def pv_chunks(kt, b):
    """[(off, w)] chunks within q-block b for k-tile kt."""
    off = max(0, kt * P - b * 512)
    return [(off, 512 - off)]


def emit_consts(nc, tc, const, ba_d, bp_d):
    from concourse.masks import make_identity
    ident = const.tile([P, P], F32, tag="ident")
    make_identity(nc, ident)
    tri = const.tile([P, P], BF16, tag="tri")
    make_upper_triangular(nc, tri, val=1.0, diag=True)
    ones_bf = const.tile([P, P], BF16, tag="ones")
    nc.gpsimd.memset(ones_bf[:], 1.0)
    bias_qk = const.tile([P, 12], F32, tag="bqk")
    nc.sync.dma_start(bias_qk[:], ba_d[0 : 2 * C].rearrange("(o p) -> p o", p=P))
    Bv = const.tile([P, C], F32, tag="Bv")
    nc.sync.dma_start(
        Bv[:],
        ba_d[2 * C : 3 * C].rearrange("(a j) -> a j", a=1).to_broadcast([P, C]),
    )
    Bp = const.tile([P, C], F32, tag="Bp")
    nc.sync.dma_start(
        Bp[:], bp_d.rearrange("(a j) -> a j", a=1).to_broadcast([P, C])
    )
    return dict(ident=ident, tri=tri, ones_bf=ones_bf, bias_qk=bias_qk, Bv=Bv, Bp=Bp)


def emit_body(nc, tc, pools, cst, dram, x_d, wa_d, wp_d, y_d):
    const, persist, nat, work, ptp, ps512, ps384 = pools
    tri, ones_bf = cst["tri"], cst["ones_bf"]
    bias_qk, Bv, Bp = cst["bias_qk"], cst["Bv"], cst["Bp"]

    xT = persist.tile([P, CT, T], BF16, tag="xT")
    WT = persist.tile([P, CT, 3 * C], BF16, tag="WT")
    WpT = persist.tile([P, CT, C], BF16, tag="WpT")
    qT = persist.tile([P, CT, T], BF16, tag="qT")         # Q features, pair layout
    KTp = persist.tile([P, 12, T], BF16, tag="KTp")       # per-head, zero-padded
    V = persist.tile([P, TT, 12, 65], BF16, tag="Vaug")
    yT = persist.tile([P, CT, T], BF16, tag="yT")

    # ---- phase 0: transpose x, W_attn, W_proj via PE (identity matmul)
    ident = cst["ident"]

    def transpose_in(dram_src, n_row_tiles, dst):
        src = dram_src.rearrange("(rt p) c -> rt p c", p=P)
        for rt in range(n_row_tiles):
            natt = nat.tile([P, C], F32, tag="nat")
            nc.sync.dma_start(natt[:], src[rt])
            for g in range(2):
                ps = ps512.tile([P, 512], F32, tag="ps512", name="tps")[:, :384]
                for i in range(3):
                    ct = g * 3 + i
                    nc.tensor.transpose(
                        ps[:, i * P : (i + 1) * P],
                        natt[:, ct * P : (ct + 1) * P],
                        ident[:],
                    )
                eng = nc.vector if rt % 2 == 0 else nc.scalar
                dstap = dst[:, g * 3 : (g + 1) * 3, rt * P : (rt + 1) * P]
                srcap = ps[:].rearrange("p (a b) -> p a b", a=3)
                if eng is nc.vector:
                    nc.vector.tensor_copy(dstap, srcap)
                else:
                    nc.scalar.activation(
                        dstap, srcap, mybir.ActivationFunctionType.Copy
                    )

    transpose_in(wa_d, 18, WT)
    transpose_in(x_d, TT, xT)
    transpose_in(wp_d, CT, WpT)

    # ---- phase 1: Q -> qT (pair layout), K -> KTp (padded per head), V [t, j]
    for jt in range(12):
        pss = [ps512.tile([P, 512], F32, tag="ps512", name=f"qkps{i}") for i in range(2)]
        for ct in range(CT):
            for tb in range(2):
                nc.tensor.matmul(
                    pss[tb][:],
                    WT[:, ct, jt * P : (jt + 1) * P],
                    xT[:, ct, tb * 512 : (tb + 1) * 512],
                    start=(ct == 0),
                    stop=(ct == CT - 1),
                )
        for tb in range(2):
            sl = slice(tb * 512, (tb + 1) * 512)
            if jt < 6:
                nc.vector.tensor_scalar_add(
                    qT[:, jt, sl], pss[tb][:], bias_qk[:, jt : jt + 1]
                )
            else:
                j = jt - 6
                nc.vector.tensor_scalar_add(
                    KTp[0:64, 2 * j, sl], pss[tb][0:64, :],
                    bias_qk[0:64, jt : jt + 1],
                )
                nc.vector.tensor_scalar_add(
                    KTp[64:128, 2 * j + 1, sl], pss[tb][64:128, :],
                    bias_qk[64:128, jt : jt + 1],
                )
    for tt in range(TT):
        pss = [ps384.tile([P, 384], F32, tag="ps384", name=f"vps{i}") for i in range(2)]
        for ct in range(CT):
            for jb in range(2):
                nc.tensor.matmul(
                    pss[jb][:],
                    xT[:, ct, tt * P : (tt + 1) * P],
                    WT[:, ct, 2 * C + jb * 384 : 2 * C + (jb + 1) * 384],
                    start=(ct == 0),
                    stop=(ct == CT - 1),
                )
        for jb in range(2):
            sl = slice(jb * 384, (jb + 1) * 384)
            nc.vector.tensor_add(
                V[:, tt, 6 * jb : 6 * jb + 6, 0:64],
                pss[jb][:].rearrange("p (h d) -> p h d", d=64),
                Bv[:, sl].rearrange("p (h d) -> p h d", d=64),
            )

    # ---- phase 2: per-head attention
    for h in range(H):
        hb = 64 * (h % 2)       # partition base of this head's rows
        vb = hb                 # psum col base for PV output
        db = 64 - hb            # psum col base for denominator
        qj = h // 2

        PT = ptp.tile([P, KT, T], BF16, tag="PT")

        def emit_qk(kt):
            qs = kt * P
            for (q0, w) in qk_chunks(kt):
                sps = ps512.tile([P, 512], F32, tag="ps512")
                nc.tensor.matmul(
                    sps[:, :w],
                    KTp[:, h, kt * P : (kt + 1) * P],
                    qT[:, qj, q0 : q0 + w],
                    start=True,
                    stop=True,
                )
                nc.scalar.activation(
                    PT[:, kt, q0 : q0 + w],
                    sps[:, :w],
                    mybir.ActivationFunctionType.Exp,
                    scale=0.125,
                )
            nc.gpsimd.affine_select(
                out=PT[:, kt, qs : qs + P],
                in_=PT[:, kt, qs : qs + P],
                compare_op=mybir.AluOpType.is_ge,
                fill=0.0,
                base=0,
                # keep where (-k + q') >= 0, else fill 0
                pattern=[[1, P]],
                channel_multiplier=-1,
            )

        def emit_pv(b):
            yD = ps512.tile([P, 512], F32, tag="ps512")
            n_kt = 4 if b == 0 else 8
            for kt in range(n_kt):
                for (off, w) in pv_chunks(kt, b):
                    first = kt == 0
                    last = kt == n_kt - 1
                    rhs = PT[:, kt, b * 512 + off : b * 512 + off + w]
                    nc.tensor.matmul(
                        yD[0:65, off : off + w],
                        V[:, kt, h, :],
                        rhs,
                        start=first,
                        stop=last,
                    )
            Dr = work.tile([P, 512], F32, tag="Dr")
            nc.vector.reciprocal(Dr[64:65, :], yD[64:65, :])
            Drb = work.tile([P, 512], BF16, tag="Drb")
            nc.vector.tensor_copy(Drb[64:65, :], Dr[64:65, :])
            rps = ps512.tile([P, 512], F32, tag="ps512")
            nc.tensor.matmul(
                rps[0:64, :],
                ones_bf[64:65, 0:64],
                Drb[64:65, :],
                start=True,
                stop=True,
                tile_position=(64, 0),
            )
            Rh = work.tile([P, 512], F32, tag="Rh")
            nc.scalar.activation(
                Rh[0:64, :],
                rps[0:64, :],
                mybir.ActivationFunctionType.Copy,
            )
            if hb == 0:
                nc.vector.tensor_mul(
                    yT[0:64, h // 2, b * 512 : (b + 1) * 512],
                    yD[0:64, :],
                    Rh[0:64, :],
                )
            else:
                yTt = work.tile([P, 512], BF16, tag="yTt")
                nc.vector.tensor_mul(yTt[0:64, :], yD[0:64, :], Rh[0:64, :])
                nc.sync.dma_start(
                    yT[64:128, h // 2, b * 512 : (b + 1) * 512], yTt[0:64, :]
                )

        for kt in range(4):
            emit_qk(kt)
        emit_pv(0)
        for kt in range(4, 8):
            emit_qk(kt)
        emit_pv(1)

    # ---- phase 3: out = y @ Wp^T + b_p
    for tt in range(TT):
        osb = work.tile([P, C], F32, tag="osb")
        pss = [ps384.tile([P, 384], F32, tag="ps384", name=f"vps{i}") for i in range(2)]
        for ct in range(CT):
            for jb in range(2):
                nc.tensor.matmul(
                    pss[jb][:],
                    yT[:, ct, tt * P : (tt + 1) * P],
                    WpT[:, ct, jb * 384 : (jb + 1) * 384],
                    start=(ct == 0),
                    stop=(ct == CT - 1),
                )
        for jb in range(2):
            sl = slice(jb * 384, (jb + 1) * 384)
            nc.vector.tensor_add(osb[:, sl], pss[jb][:], Bp[:, sl])
        nc.sync.dma_start(
            y_d.rearrange("(tt p) c -> tt p c", p=P)[tt], osb[:]
        )


def build_program(loop=1):
    nc = bacc.Bacc("TRN2", target_bir_lowering=False, debug=False)
    x_d = nc.dram_tensor("x", [T, C], F32, kind="ExternalInput").ap()
    wa_d = nc.dram_tensor("W_attn", [3 * C, C], F32, kind="ExternalInput").ap()
    ba_d = nc.dram_tensor("b_attn", [3 * C], F32, kind="ExternalInput").ap()
    wp_d = nc.dram_tensor("W_proj", [C, C], F32, kind="ExternalInput").ap()
    bp_d = nc.dram_tensor("b_proj", [C], F32, kind="ExternalInput").ap()
    y_d = nc.dram_tensor("y", [T, C], F32, kind="ExternalOutput").ap()

    with tile.TileContext(nc) as tc, ExitStack() as ctx:
        const = ctx.enter_context(tc.tile_pool(name="const", bufs=1))
        persist = ctx.enter_context(tc.tile_pool(name="persist", bufs=1))
        nat = ctx.enter_context(tc.tile_pool(name="nat", bufs=4))
        work = ctx.enter_context(tc.tile_pool(name="work", bufs=2))
        ptp = ctx.enter_context(tc.tile_pool(name="ptp", bufs=2))
        dram = ctx.enter_context(tc.tile_pool(name="dram", bufs=1, space="DRAM"))
        ps512 = ctx.enter_context(tc.tile_pool(name="ps512", bufs=6, space="PSUM"))
        ps384 = ctx.enter_context(tc.tile_pool(name="ps384", bufs=2, space="PSUM"))
        pools = (const, persist, nat, work, ptp, ps512, ps384)

        cst = emit_consts(nc, tc, const, ba_d, bp_d)
        # zero-fill KTp's complementary halves + V ones columns once
        KTp0 = persist.tile([P, 12, T], BF16, tag="KTp")
        nc.gpsimd.memset(KTp0[:], 0.0)
        V0 = persist.tile([P, TT, 12, 65], BF16, tag="Vaug")
        nc.gpsimd.memset(V0[:, :, :, 64:65], 1.0)
        loop_cm = tc.For_i(0, loop, 1) if loop > 1 else contextlib.nullcontext()
        with loop_cm:
            emit_body(nc, tc, pools, cst, dram, x_d, wa_d, wp_d, y_d)

    nc.compile()
    return nc


_CACHED_NC = None


def kernel(x, W_attn, b_attn, W_proj, b_proj):
    from concourse.bass_utils import run_bass_kernel_spmd

    global _CACHED_NC
    if _CACHED_NC is None:
        _CACHED_NC = build_program(loop=1)
    nc = _CACHED_NC

    B = x.shape[0]
    assert B == N_CORES
    in_maps = [
        {
            "x": np.ascontiguousarray(x[b], dtype=np.float32),
            "W_attn": np.asarray(W_attn, dtype=np.float32),
            "b_attn": np.asarray(b_attn, dtype=np.float32),
            "W_proj": np.asarray(W_proj, dtype=np.float32),
            "b_proj": np.asarray(b_proj, dtype=np.float32),
        }
        for b in range(B)
    ]
    res = run_bass_kernel_spmd(nc, in_maps, list(range(N_CORES)))
    return np.stack([res.results[b]["y"] for b in range(B)], axis=0)


# revision 13
# speedup vs baseline: 1.1050x; 1.0015x over previous
"""Causal self-attention kernel for Trainium2, 8-core data parallel.

Per-core program: one batch element b of x [8, 1024, 768].
  - x, W_attn, W_proj transposed into c-on-partitions layouts via
    cast-to-bf16 + DRAM round-trip DMA transpose.
  - qkT (Q features) / KTpad (K features, zero-padded to 128 rows for FWL)
    / V [t, j]: weight-reuse-ordered GEMMs.
  - per head: S^T = K^T.T @ Q^T (k on partitions, q free), P^T = exp(S^T/8)
    with causal tri-mask on diagonal blocks; y^T_h = V^T-slices @ P^T with a
    concurrent col-tiled ones-matmul accumulating softmax denominators;
    normalized via PE-broadcast reciprocal.
  - out = y @ Wp^T + b_p (t on partitions).
All matmuls bf16 inputs / fp32 PSUM accumulation; softmax in fp32.
"""
import sys
import contextlib
from contextlib import ExitStack

sys.path.insert(0, "/opt/trn_rl_repo")

import numpy as np

import concourse.bass as bass
import concourse.bacc as bacc
import concourse.mybir as mybir
import concourse.tile as tile
from concourse.masks import make_upper_triangular

F32 = mybir.dt.float32
BF16 = mybir.dt.bfloat16

P = 128
T = 1024
C = 768
H = 12
HS = 64
CT = C // P     # 6 c-tiles
TT = T // P     # 8 t-tiles
KT = T // P     # 8 k-tiles per head
N_CORES = 8


def qk_chunks(kt):
    """[(q0, w)] matmul chunks for k-tile kt: q >= kt*128, each split in two
    for stationary-weight reuse."""
    qs = kt * P
    if kt < 4:
        return [(qs, 512 - qs), (512, 512)]
    return [(qs, T - qs)]


def pv_chunks(kt, b):
    """[(off, w)] chunks within q-block b for k-tile kt."""
    off = max(0, kt * P - b * 512)
    return [(off, 512 - off)]


def emit_consts(nc, tc, const, ba_d, bp_d):
    from concourse.masks import make_identity
    ident = const.tile([P, P], F32, tag="ident")
    make_identity(nc, ident)
    tri = const.tile([P, P], BF16, tag="tri")
    make_upper_triangular(nc, tri, val=1.0, diag=True)
    ones_bf = const.tile([P, P], BF16, tag="ones")
    nc.gpsimd.memset(ones_bf[:], 1.0)
    bias_qk = const.tile([P, 12], F32, tag="bqk")
    nc.sync.dma_start(bias_qk[:], ba_d[0 : 2 * C].rearrange("(o p) -> p o", p=P))
    Bv = const.tile([P, C], F32, tag="Bv")
    nc.sync.dma_start(
        Bv[:],
        ba_d[2 * C : 3 * C].rearrange("(a j) -> a j", a=1).to_broadcast([P, C]),
    )
    Bp = const.tile([P, C], F32, tag="Bp")
    nc.sync.dma_start(
        Bp[:], bp_d.rearrange("(a j) -> a j", a=1).to_broadcast([P, C])
    )
    return dict(ident=ident, tri=tri, ones_bf=ones_bf, bias_qk=bias_qk, Bv=Bv, Bp=Bp)


def emit_body(nc, tc, pools, cst, dram, x_d, wa_d, wp_d, y_d):
    const, persist, nat, work, ptp, ps512, ps384 = pools
    tri, ones_bf = cst["tri"], cst["ones_bf"]
    bias_qk, Bv, Bp = cst["bias_qk"], cst["Bv"], cst["Bp"]

    xT = persist.tile([P, CT, T], BF16, tag="xT")
    WT = persist.tile([P, CT, 3 * C], BF16, tag="WT")
    WpT = persist.tile([P, CT, C], BF16, tag="WpT")
    qT = persist.tile([P, CT, T], BF16, tag="qT")         # Q features, pair layout
    KTp = persist.tile([P, 12, T], BF16, tag="KTp")       # per-head, zero-padded
    V = persist.tile([P, TT, 12, 65], BF16, tag="Vaug")
    yT = persist.tile([P, CT, T], BF16, tag="yT")

    # ---- phase 0: transpose x, W_attn, W_proj via PE (identity matmul)
    ident = cst["ident"]

    def transpose_in(dram_src, n_row_tiles, dst):
        src = dram_src.rearrange("(rt p) c -> rt p c", p=P)
        for rt in range(n_row_tiles):
            natt = nat.tile([P, C], F32, tag="nat")
            nc.sync.dma_start(natt[:], src[rt])
            for g in range(2):
                ps = ps384.tile([P, 384], F32, tag="ps384")
                for i in range(3):
                    ct = g * 3 + i
                    nc.tensor.transpose(
                        ps[:, i * P : (i + 1) * P],
                        natt[:, ct * P : (ct + 1) * P],
                        ident[:],
                    )
                eng = nc.vector if rt % 2 == 0 else nc.scalar
                dstap = dst[:, g * 3 : (g + 1) * 3, rt * P : (rt + 1) * P]
                srcap = ps[:].rearrange("p (a b) -> p a b", a=3)
                if eng is nc.vector:
                    nc.vector.tensor_copy(dstap, srcap)
                else:
                    nc.scalar.activation(
                        dstap, srcap, mybir.ActivationFunctionType.Copy
                    )

    transpose_in(wa_d, 18, WT)
    transpose_in(x_d, TT, xT)
    transpose_in(wp_d, CT, WpT)

    # ---- phase 1: Q -> qT (pair layout), K -> KTp (padded per head), V [t, j]
    for jt in range(12):
        pss = [ps512.tile([P, 512], F32, tag="ps512", name=f"qkps{i}") for i in range(2)]
        for ct in range(CT):
            for tb in range(2):
                nc.tensor.matmul(
                    pss[tb][:],
                    WT[:, ct, jt * P : (jt + 1) * P],
                    xT[:, ct, tb * 512 : (tb + 1) * 512],
                    start=(ct == 0),
                    stop=(ct == CT - 1),
                )
        for tb in range(2):
            sl = slice(tb * 512, (tb + 1) * 512)
            if jt < 6:
                nc.vector.tensor_scalar_add(
                    qT[:, jt, sl], pss[tb][:], bias_qk[:, jt : jt + 1]
                )
            else:
                j = jt - 6
                nc.vector.tensor_scalar_add(
                    KTp[0:64, 2 * j, sl], pss[tb][0:64, :],
                    bias_qk[0:64, jt : jt + 1],
                )
                nc.vector.tensor_scalar_add(
                    KTp[64:128, 2 * j + 1, sl], pss[tb][64:128, :],
                    bias_qk[64:128, jt : jt + 1],
                )
    for tt in range(TT):
        pss = [ps384.tile([P, 384], F32, tag="ps384", name=f"vps{i}") for i in range(2)]
        for ct in range(CT):
            for jb in range(2):
                nc.tensor.matmul(
                    pss[jb][:],
                    xT[:, ct, tt * P : (tt + 1) * P],
                    WT[:, ct, 2 * C + jb * 384 : 2 * C + (jb + 1) * 384],
                    start=(ct == 0),
                    stop=(ct == CT - 1),
                )
        for jb in range(2):
            sl = slice(jb * 384, (jb + 1) * 384)
            nc.vector.tensor_add(
                V[:, tt, 6 * jb : 6 * jb + 6, 0:64],
                pss[jb][:].rearrange("p (h d) -> p h d", d=64),
                Bv[:, sl].rearrange("p (h d) -> p h d", d=64),
            )

    # ---- phase 2: per-head attention
    for h in range(H):
        hb = 64 * (h % 2)       # partition base of this head's rows
        vb = hb                 # psum col base for PV output
        db = 64 - hb            # psum col base for denominator
        qj = h // 2

        PT = ptp.tile([P, KT, T], BF16, tag="PT")
        for kt in range(KT):
            qs = kt * P
            for (q0, w) in qk_chunks(kt):
                sps = ps512.tile([P, 512], F32, tag="ps512")
                nc.tensor.matmul(
                    sps[:, :w],
                    KTp[:, h, kt * P : (kt + 1) * P],
                    qT[:, qj, q0 : q0 + w],
                    start=True,
                    stop=True,
                )
                nc.scalar.activation(
                    PT[:, kt, q0 : q0 + w],
                    sps[:, :w],
                    mybir.ActivationFunctionType.Exp,
                    scale=0.125,
                )
            nc.gpsimd.affine_select(
                out=PT[:, kt, qs : qs + P],
                in_=PT[:, kt, qs : qs + P],
                compare_op=mybir.AluOpType.is_ge,
                fill=0.0,
                base=0,
                # keep where (-k + q') >= 0, else fill 0
                pattern=[[1, P]],
                channel_multiplier=-1,
            )

        for b in range(2):
            yD = ps512.tile([P, 512], F32, tag="ps512")
            n_kt = 4 if b == 0 else 8
            for kt in range(n_kt):
                for (off, w) in pv_chunks(kt, b):
                    first = kt == 0
                    last = kt == n_kt - 1
                    rhs = PT[:, kt, b * 512 + off : b * 512 + off + w]
                    nc.tensor.matmul(
                        yD[0:65, off : off + w],
                        V[:, kt, h, :],
                        rhs,
                        start=first,
                        stop=last,
                    )
            Dr = work.tile([P, 512], F32, tag="Dr")
            nc.vector.reciprocal(Dr[64:65, :], yD[64:65, :])
            Drb = work.tile([P, 512], BF16, tag="Drb")
            nc.vector.tensor_copy(Drb[64:65, :], Dr[64:65, :])
            rps = ps512.tile([P, 512], F32, tag="ps512")
            nc.tensor.matmul(
                rps[0:64, :],
                ones_bf[64:65, 0:64],
                Drb[64:65, :],
                start=True,
                stop=True,
                tile_position=(64, 0),
            )
            Rh = work.tile([P, 512], F32, tag="Rh")
            nc.scalar.activation(
                Rh[0:64, :],
                rps[0:64, :],
                mybir.ActivationFunctionType.Copy,
            )
            if hb == 0:
                nc.vector.tensor_mul(
                    yT[0:64, h // 2, b * 512 : (b + 1) * 512],
                    yD[0:64, :],
                    Rh[0:64, :],
                )
            else:
                yTt = work.tile([P, 512], BF16, tag="yTt")
                nc.vector.tensor_mul(yTt[0:64, :], yD[0:64, :], Rh[0:64, :])
                nc.sync.dma_start(
                    yT[64:128, h // 2, b * 512 : (b + 1) * 512], yTt[0:64, :]
                )

    # ---- phase 3: out = y @ Wp^T + b_p
    for tt in range(TT):
        osb = work.tile([P, C], F32, tag="osb")
        pss = [ps384.tile([P, 384], F32, tag="ps384", name=f"vps{i}") for i in range(2)]
        for ct in range(CT):
            for jb in range(2):
                nc.tensor.matmul(
                    pss[jb][:],
                    yT[:, ct, tt * P : (tt + 1) * P],
                    WpT[:, ct, jb * 384 : (jb + 1) * 384],
                    start=(ct == 0),
                    stop=(ct == CT - 1),
                )
        for jb in range(2):
            sl = slice(jb * 384, (jb + 1) * 384)
            nc.vector.tensor_add(osb[:, sl], pss[jb][:], Bp[:, sl])
        nc.sync.dma_start(
            y_d.rearrange("(tt p) c -> tt p c", p=P)[tt], osb[:]
        )


def build_program(loop=1):
    nc = bacc.Bacc("TRN2", target_bir_lowering=False, debug=False)
    x_d = nc.dram_tensor("x", [T, C], F32, kind="ExternalInput").ap()
    wa_d = nc.dram_tensor("W_attn", [3 * C, C], F32, kind="ExternalInput").ap()
    ba_d = nc.dram_tensor("b_attn", [3 * C], F32, kind="ExternalInput").ap()
    wp_d = nc.dram_tensor("W_proj", [C, C], F32, kind="ExternalInput").ap()
    bp_d = nc.dram_tensor("b_proj", [C], F32, kind="ExternalInput").ap()
    y_d = nc.dram_tensor("y", [T, C], F32, kind="ExternalOutput").ap()

    with tile.TileContext(nc) as tc, ExitStack() as ctx:
        const = ctx.enter_context(tc.tile_pool(name="const", bufs=1))
        persist = ctx.enter_context(tc.tile_pool(name="persist", bufs=1))
        nat = ctx.enter_context(tc.tile_pool(name="nat", bufs=6))
        work = ctx.enter_context(tc.tile_pool(name="work", bufs=2))
        ptp = ctx.enter_context(tc.tile_pool(name="ptp", bufs=3))
        dram = ctx.enter_context(tc.tile_pool(name="dram", bufs=1, space="DRAM"))
        ps512 = ctx.enter_context(tc.tile_pool(name="ps512", bufs=6, space="PSUM"))
        ps384 = ctx.enter_context(tc.tile_pool(name="ps384", bufs=2, space="PSUM"))
        pools = (const, persist, nat, work, ptp, ps512, ps384)

        cst = emit_consts(nc, tc, const, ba_d, bp_d)
        # zero-fill KTp's complementary halves + V ones columns once
        KTp0 = persist.tile([P, 12, T], BF16, tag="KTp")
        nc.gpsimd.memset(KTp0[:], 0.0)
        V0 = persist.tile([P, TT, 12, 65], BF16, tag="Vaug")
        nc.gpsimd.memset(V0[:, :, :, 64:65], 1.0)
        loop_cm = tc.For_i(0, loop, 1) if loop > 1 else contextlib.nullcontext()
        with loop_cm:
            emit_body(nc, tc, pools, cst, dram, x_d, wa_d, wp_d, y_d)

    nc.compile()
    return nc


_CACHED_NC = None


def kernel(x, W_attn, b_attn, W_proj, b_proj):
    from concourse.bass_utils import run_bass_kernel_spmd

    global _CACHED_NC
    if _CACHED_NC is None:
        _CACHED_NC = build_program(loop=1)
    nc = _CACHED_NC

    B = x.shape[0]
    assert B == N_CORES
    in_maps = [
        {
            "x": np.ascontiguousarray(x[b], dtype=np.float32),
            "W_attn": np.asarray(W_attn, dtype=np.float32),
            "b_attn": np.asarray(b_attn, dtype=np.float32),
            "W_proj": np.asarray(W_proj, dtype=np.float32),
            "b_proj": np.asarray(b_proj, dtype=np.float32),
        }
        for b in range(B)
    ]
    res = run_bass_kernel_spmd(nc, in_maps, list(range(N_CORES)))
    return np.stack([res.results[b]["y"] for b in range(B)], axis=0)


# revision 15
# speedup vs baseline: 1.1065x; 1.0013x over previous
"""Causal self-attention kernel for Trainium2, 8-core data parallel.

Per-core program: one batch element b of x [8, 1024, 768].
  - x, W_attn, W_proj transposed into c-on-partitions layouts via
    cast-to-bf16 + DRAM round-trip DMA transpose.
  - qkT (Q features) / KTpad (K features, zero-padded to 128 rows for FWL)
    / V [t, j]: weight-reuse-ordered GEMMs.
  - per head: S^T = K^T.T @ Q^T (k on partitions, q free), P^T = exp(S^T/8)
    with causal tri-mask on diagonal blocks; y^T_h = V^T-slices @ P^T with a
    concurrent col-tiled ones-matmul accumulating softmax denominators;
    normalized via PE-broadcast reciprocal.
  - out = y @ Wp^T + b_p (t on partitions).
All matmuls bf16 inputs / fp32 PSUM accumulation; softmax in fp32.
"""
import sys
import contextlib
from contextlib import ExitStack

sys.path.insert(0, "/opt/trn_rl_repo")

import numpy as np

import concourse.bass as bass
import concourse.bacc as bacc
import concourse.mybir as mybir
import concourse.tile as tile
from concourse.masks import make_upper_triangular

F32 = mybir.dt.float32
BF16 = mybir.dt.bfloat16

P = 128
T = 1024
C = 768
H = 12
HS = 64
CT = C // P     # 6 c-tiles
TT = T // P     # 8 t-tiles
KT = T // P     # 8 k-tiles per head
N_CORES = 8


def qk_chunks(kt):
    """[(q0, w)] matmul chunks for k-tile kt: q >= kt*128, each split in two
    for stationary-weight reuse."""
    qs = kt * P
    if kt < 4:
        return [(qs, 512 - qs), (512, 512)]
    return [(qs, T - qs)]


def pv_chunks(kt, b):
    """[(off, w)] chunks within q-block b for k-tile kt."""
    off = max(0, kt * P - b * 512)
    return [(off, 512 - off)]


def emit_consts(nc, tc, const, ba_d, bp_d):
    from concourse.masks import make_identity
    ident = const.tile([P, P], F32, tag="ident")
    make_identity(nc, ident)
    tri = const.tile([P, P], BF16, tag="tri")
    make_upper_triangular(nc, tri, val=1.0, diag=True)
    ones_bf = const.tile([P, P], BF16, tag="ones")
    nc.gpsimd.memset(ones_bf[:], 1.0)
    bias_qk = const.tile([P, 12], F32, tag="bqk")
    nc.sync.dma_start(bias_qk[:], ba_d[0 : 2 * C].rearrange("(o p) -> p o", p=P))
    Bv = const.tile([P, C], F32, tag="Bv")
    nc.sync.dma_start(
        Bv[:],
        ba_d[2 * C : 3 * C].rearrange("(a j) -> a j", a=1).to_broadcast([P, C]),
    )
    Bp = const.tile([P, C], F32, tag="Bp")
    nc.sync.dma_start(
        Bp[:], bp_d.rearrange("(a j) -> a j", a=1).to_broadcast([P, C])
    )
    return dict(ident=ident, tri=tri, ones_bf=ones_bf, bias_qk=bias_qk, Bv=Bv, Bp=Bp)


def emit_body(nc, tc, pools, cst, dram, x_d, wa_d, wp_d, y_d):
    const, persist, nat, work, ptp, ps512, ps384 = pools
    tri, ones_bf = cst["tri"], cst["ones_bf"]
    bias_qk, Bv, Bp = cst["bias_qk"], cst["Bv"], cst["Bp"]

    xT = persist.tile([P, CT, T], BF16, tag="xT")
    WT = persist.tile([P, CT, 3 * C], BF16, tag="WT")
    WpT = persist.tile([P, CT, C], BF16, tag="WpT")
    qT = persist.tile([P, CT, T], BF16, tag="qT")         # Q features, pair layout
    KTp = persist.tile([P, 12, T], BF16, tag="KTp")       # per-head, zero-padded
    V = persist.tile([P, TT, 12, 65], BF16, tag="Vaug")
    yT = persist.tile([P, CT, T], BF16, tag="yT")

    # ---- phase 0: transpose x, W_attn, W_proj via PE (identity matmul)
    ident = cst["ident"]

    def transpose_in(dram_src, n_row_tiles, dst):
        src = dram_src.rearrange("(rt p) c -> rt p c", p=P)
        for rt in range(n_row_tiles):
            natt = nat.tile([P, C], F32, tag="nat")
            nc.sync.dma_start(natt[:], src[rt])
            for g in range(2):
                ps = ps512.tile([P, 512], F32, tag="ps512", name="tps")[:, :384]
                for i in range(3):
                    ct = g * 3 + i
                    nc.tensor.transpose(
                        ps[:, i * P : (i + 1) * P],
                        natt[:, ct * P : (ct + 1) * P],
                        ident[:],
                    )
                eng = nc.vector if rt % 2 == 0 else nc.scalar
                dstap = dst[:, g * 3 : (g + 1) * 3, rt * P : (rt + 1) * P]
                srcap = ps[:].rearrange("p (a b) -> p a b", a=3)
                if eng is nc.vector:
                    nc.vector.tensor_copy(dstap, srcap)
                else:
                    nc.scalar.activation(
                        dstap, srcap, mybir.ActivationFunctionType.Copy
                    )

    transpose_in(x_d, TT, xT)
    transpose_in(wa_d, 18, WT)
    transpose_in(wp_d, CT, WpT)

    # ---- phase 1: Q -> qT (pair layout), K -> KTp (padded per head), V [t, j]
    for jt in range(12):
        pss = [ps512.tile([P, 512], F32, tag="ps512", name=f"qkps{i}") for i in range(2)]
        for ct in range(CT):
            for tb in range(2):
                nc.tensor.matmul(
                    pss[tb][:],
                    WT[:, ct, jt * P : (jt + 1) * P],
                    xT[:, ct, tb * 512 : (tb + 1) * 512],
                    start=(ct == 0),
                    stop=(ct == CT - 1),
                )
        for tb in range(2):
            sl = slice(tb * 512, (tb + 1) * 512)
            if jt < 6:
                nc.vector.tensor_scalar_add(
                    qT[:, jt, sl], pss[tb][:], bias_qk[:, jt : jt + 1]
                )
            else:
                j = jt - 6
                nc.vector.tensor_scalar_add(
                    KTp[0:64, 2 * j, sl], pss[tb][0:64, :],
                    bias_qk[0:64, jt : jt + 1],
                )
                nc.vector.tensor_scalar_add(
                    KTp[64:128, 2 * j + 1, sl], pss[tb][64:128, :],
                    bias_qk[64:128, jt : jt + 1],
                )
    for tt in range(TT):
        pss = [ps512.tile([P, 512], F32, tag="ps512", name=f"vps{i}")[:, :384] for i in range(2)]
        for ct in range(CT):
            for jb in range(2):
                nc.tensor.matmul(
                    pss[jb][:],
                    xT[:, ct, tt * P : (tt + 1) * P],
                    WT[:, ct, 2 * C + jb * 384 : 2 * C + (jb + 1) * 384],
                    start=(ct == 0),
                    stop=(ct == CT - 1),
                )
        for jb in range(2):
            sl = slice(jb * 384, (jb + 1) * 384)
            nc.vector.tensor_add(
                V[:, tt, 6 * jb : 6 * jb + 6, 0:64],
                pss[jb][:].rearrange("p (h d) -> p h d", d=64),
                Bv[:, sl].rearrange("p (h d) -> p h d", d=64),
            )

    # ---- phase 2: per-head attention
    for h in range(H):
        hb = 64 * (h % 2)       # partition base of this head's rows
        vb = hb                 # psum col base for PV output
        db = 64 - hb            # psum col base for denominator
        qj = h // 2

        PT = ptp.tile([P, KT, T], BF16, tag="PT")
        for kt in range(KT):
            qs = kt * P
            for (q0, w) in qk_chunks(kt):
                sps = ps512.tile([P, 512], F32, tag="ps512")
                nc.tensor.matmul(
                    sps[:, :w],
                    KTp[:, h, kt * P : (kt + 1) * P],
                    qT[:, qj, q0 : q0 + w],
                    start=True,
                    stop=True,
                )
                nc.scalar.activation(
                    PT[:, kt, q0 : q0 + w],
                    sps[:, :w],
                    mybir.ActivationFunctionType.Exp,
                    scale=0.125,
                )
            nc.gpsimd.affine_select(
                out=PT[:, kt, qs : qs + P],
                in_=PT[:, kt, qs : qs + P],
                compare_op=mybir.AluOpType.is_ge,
                fill=0.0,
                base=0,
                # keep where (-k + q') >= 0, else fill 0
                pattern=[[1, P]],
                channel_multiplier=-1,
            )

        for b in range(2):
            yD = ps512.tile([P, 512], F32, tag="ps512")
            n_kt = 4 if b == 0 else 8
            for kt in range(n_kt):
                for (off, w) in pv_chunks(kt, b):
                    first = kt == 0
                    last = kt == n_kt - 1
                    rhs = PT[:, kt, b * 512 + off : b * 512 + off + w]
                    nc.tensor.matmul(
                        yD[0:65, off : off + w],
                        V[:, kt, h, :],
                        rhs,
                        start=first,
                        stop=last,
                    )
            Dr = work.tile([P, 512], F32, tag="Dr")
            nc.vector.reciprocal(Dr[64:65, :], yD[64:65, :])
            Drb = work.tile([P, 512], BF16, tag="Drb")
            nc.vector.tensor_copy(Drb[64:65, :], Dr[64:65, :])
            rps = ps512.tile([P, 512], F32, tag="ps512")
            nc.tensor.matmul(
                rps[0:64, :],
                ones_bf[64:65, 0:64],
                Drb[64:65, :],
                start=True,
                stop=True,
                tile_position=(64, 0),
            )
            Rh = work.tile([P, 512], F32, tag="Rh")
            nc.scalar.activation(
                Rh[0:64, :],
                rps[0:64, :],
                mybir.ActivationFunctionType.Copy,
            )
            if hb == 0:
                nc.vector.tensor_mul(
                    yT[0:64, h // 2, b * 512 : (b + 1) * 512],
                    yD[0:64, :],
                    Rh[0:64, :],
                )
            else:
                yTt = work.tile([P, 512], BF16, tag="yTt")
                nc.vector.tensor_mul(yTt[0:64, :], yD[0:64, :], Rh[0:64, :])
                nc.sync.dma_start(
                    yT[64:128, h // 2, b * 512 : (b + 1) * 512], yTt[0:64, :]
                )

    # ---- phase 3: out = y @ Wp^T + b_p
    for tt in range(TT):
        osb = work.tile([P, C], F32, tag="osb")
        pss = [ps512.tile([P, 512], F32, tag="ps512", name=f"vps{i}")[:, :384] for i in range(2)]
        for ct in range(CT):
            for jb in range(2):
                nc.tensor.matmul(
                    pss[jb][:],
                    yT[:, ct, tt * P : (tt + 1) * P],
                    WpT[:, ct, jb * 384 : (jb + 1) * 384],
                    start=(ct == 0),
                    stop=(ct == CT - 1),
                )
        for jb in range(2):
            sl = slice(jb * 384, (jb + 1) * 384)
            nc.vector.tensor_add(osb[:, sl], pss[jb][:], Bp[:, sl])
        nc.sync.dma_start(
            y_d.rearrange("(tt p) c -> tt p c", p=P)[tt], osb[:]
        )


def build_program(loop=1):
    nc = bacc.Bacc("TRN2", target_bir_lowering=False, debug=False)
    x_d = nc.dram_tensor("x", [T, C], F32, kind="ExternalInput").ap()
    wa_d = nc.dram_tensor("W_attn", [3 * C, C], F32, kind="ExternalInput").ap()
    ba_d = nc.dram_tensor("b_attn", [3 * C], F32, kind="ExternalInput").ap()
    wp_d = nc.dram_tensor("W_proj", [C, C], F32, kind="ExternalInput").ap()
    bp_d = nc.dram_tensor("b_proj", [C], F32, kind="ExternalInput").ap()
    y_d = nc.dram_tensor("y", [T, C], F32, kind="ExternalOutput").ap()

    with tile.TileContext(nc) as tc, ExitStack() as ctx:
        const = ctx.enter_context(tc.tile_pool(name="const", bufs=1))
        persist = ctx.enter_context(tc.tile_pool(name="persist", bufs=1))
        nat = ctx.enter_context(tc.tile_pool(name="nat", bufs=6))
        work = ctx.enter_context(tc.tile_pool(name="work", bufs=2))
        ptp = ctx.enter_context(tc.tile_pool(name="ptp", bufs=3))
        dram = ctx.enter_context(tc.tile_pool(name="dram", bufs=1, space="DRAM"))
        ps512 = ctx.enter_context(tc.tile_pool(name="ps512", bufs=8, space="PSUM"))
        ps384 = ps512
        pools = (const, persist, nat, work, ptp, ps512, ps384)

        cst = emit_consts(nc, tc, const, ba_d, bp_d)
        # zero-fill KTp's complementary halves + V ones columns once
        KTp0 = persist.tile([P, 12, T], BF16, tag="KTp")
        nc.gpsimd.memset(KTp0[:], 0.0)
        V0 = persist.tile([P, TT, 12, 65], BF16, tag="Vaug")
        nc.gpsimd.memset(V0[:, :, :, 64:65], 1.0)
        loop_cm = tc.For_i(0, loop, 1) if loop > 1 else contextlib.nullcontext()
        with loop_cm:
            emit_body(nc, tc, pools, cst, dram, x_d, wa_d, wp_d, y_d)

    nc.compile()
    return nc


_CACHED_NC = None


def kernel(x, W_attn, b_attn, W_proj, b_proj):
    from concourse.bass_utils import run_bass_kernel_spmd

    global _CACHED_NC
    if _CACHED_NC is None:
        _CACHED_NC = build_program(loop=1)
    nc = _CACHED_NC

    B = x.shape[0]
    assert B == N_CORES
    in_maps = [
        {
            "x": np.ascontiguousarray(x[b], dtype=np.float32),
            "W_attn": np.asarray(W_attn, dtype=np.float32),
            "b_attn": np.asarray(b_attn, dtype=np.float32),
            "W_proj": np.asarray(W_proj, dtype=np.float32),
            "b_proj": np.asarray(b_proj, dtype=np.float32),
        }
        for b in range(B)
    ]
    res = run_bass_kernel_spmd(nc, in_maps, list(range(N_CORES)))
    return np.stack([res.results[b]["y"] for b in range(B)], axis=0)


# revision 16
# speedup vs baseline: 1.1684x; 1.0559x over previous
"""Causal self-attention kernel for Trainium2, 8-core data parallel.

Per-core program: one batch element b of x [8, 1024, 768].
  - x, W_attn, W_proj transposed into c-on-partitions layouts via
    cast-to-bf16 + DRAM round-trip DMA transpose.
  - qkT (Q features) / KTpad (K features, zero-padded to 128 rows for FWL)
    / V [t, j]: weight-reuse-ordered GEMMs.
  - per head: S^T = K^T.T @ Q^T (k on partitions, q free), P^T = exp(S^T/8)
    with causal tri-mask on diagonal blocks; y^T_h = V^T-slices @ P^T with a
    concurrent col-tiled ones-matmul accumulating softmax denominators;
    normalized via PE-broadcast reciprocal.
  - out = y @ Wp^T + b_p (t on partitions).
All matmuls bf16 inputs / fp32 PSUM accumulation; softmax in fp32.
"""
import sys
import contextlib
from contextlib import ExitStack

sys.path.insert(0, "/opt/trn_rl_repo")

import numpy as np

import concourse.bass as bass
import concourse.bacc as bacc
import concourse.mybir as mybir
import concourse.tile as tile
from concourse.masks import make_upper_triangular

F32 = mybir.dt.float32
BF16 = mybir.dt.bfloat16

P = 128
T = 1024
C = 768
H = 12
HS = 64
CT = C // P     # 6 c-tiles
TT = T // P     # 8 t-tiles
KT = T // P     # 8 k-tiles per head
N_CORES = 8


def qk_chunks(kt):
    """[(q0, w)] matmul chunks for k-tile kt: q >= kt*128, each split in two
    for stationary-weight reuse."""
    qs = kt * P
    if kt < 4:
        return [(qs, 512 - qs), (512, 512)]
    return [(qs, T - qs)]


def pv_chunks(kt, b):
    """[(off, w)] chunks within q-block b for k-tile kt."""
    off = max(0, kt * P - b * 512)
    return [(off, 512 - off)]


def emit_consts(nc, tc, const, ba_d, bp_d):
    from concourse.masks import make_identity
    ident = const.tile([P, P], F32, tag="ident")
    make_identity(nc, ident)
    tri = const.tile([P, P], BF16, tag="tri")
    make_upper_triangular(nc, tri, val=1.0, diag=True)
    ones_bf = const.tile([P, P], BF16, tag="ones")
    nc.gpsimd.memset(ones_bf[:], 1.0)
    bias_qk = const.tile([P, 12], F32, tag="bqk")
    nc.sync.dma_start(bias_qk[:], ba_d[0 : 2 * C].rearrange("(o p) -> p o", p=P))
    Bv = const.tile([P, C], F32, tag="Bv")
    nc.sync.dma_start(
        Bv[:],
        ba_d[2 * C : 3 * C].rearrange("(a j) -> a j", a=1).to_broadcast([P, C]),
    )
    Bp = const.tile([P, C], F32, tag="Bp")
    nc.sync.dma_start(
        Bp[:], bp_d.rearrange("(a j) -> a j", a=1).to_broadcast([P, C])
    )
    return dict(ident=ident, tri=tri, ones_bf=ones_bf, bias_qk=bias_qk, Bv=Bv, Bp=Bp)


def emit_body(nc, tc, pools, cst, dram, x_d, wa_d, wp_d, y_d):
    const, persist, nat, work, ptp, ps512, ps384 = pools
    tri, ones_bf = cst["tri"], cst["ones_bf"]
    bias_qk, Bv, Bp = cst["bias_qk"], cst["Bv"], cst["Bp"]

    xT = persist.tile([P, CT, T], BF16, tag="xT")
    WT = persist.tile([P, CT, 3 * C], BF16, tag="WT")
    WpT = persist.tile([P, CT, C], BF16, tag="WpT")
    qT = persist.tile([P, CT, T], BF16, tag="qT")         # Q features, pair layout
    KTp = persist.tile([P, 12, T], BF16, tag="KTp")       # per-head, zero-padded
    V = persist.tile([P, TT, 12, 65], BF16, tag="Vaug")
    yT = persist.tile([P, CT, T], BF16, tag="yT")

    # ---- phase 0: transpose x, W_attn, W_proj via PE (identity matmul)
    ident = cst["ident"]

    def transpose_in(dram_src, n_row_tiles, dst):
        src = dram_src.rearrange("(rt p) c -> rt p c", p=P)
        for rt in range(n_row_tiles):
            natt = nat.tile([P, C], F32, tag="nat")
            nc.sync.dma_start(natt[:], src[rt])
            for g in range(2):
                ps = ps512.tile([P, 512], F32, tag="ps512", name="tps")[:, :384]
                for i in range(3):
                    ct = g * 3 + i
                    nc.tensor.transpose(
                        ps[:, i * P : (i + 1) * P],
                        natt[:, ct * P : (ct + 1) * P],
                        ident[:],
                    )
                eng = nc.vector if rt % 2 == 0 else nc.scalar
                dstap = dst[:, g * 3 : (g + 1) * 3, rt * P : (rt + 1) * P]
                srcap = ps[:].rearrange("p (a b) -> p a b", a=3)
                if eng is nc.vector:
                    nc.vector.tensor_copy(dstap, srcap)
                else:
                    nc.scalar.activation(
                        dstap, srcap, mybir.ActivationFunctionType.Copy
                    )

    transpose_in(x_d, TT, xT)
    transpose_in(wa_d, 18, WT)
    transpose_in(wp_d, CT, WpT)

    # ---- phase 1: Q -> qT (pair layout), K -> KTp (padded per head), V [t, j]
    for jt in range(12):
        pss = [ps512.tile([P, 512], F32, tag="ps512", name=f"qkps{i}") for i in range(2)]
        for ct in range(CT):
            for tb in range(2):
                nc.tensor.matmul(
                    pss[tb][:],
                    WT[:, ct, jt * P : (jt + 1) * P],
                    xT[:, ct, tb * 512 : (tb + 1) * 512],
                    start=(ct == 0),
                    stop=(ct == CT - 1),
                )
        for tb in range(2):
            sl = slice(tb * 512, (tb + 1) * 512)
            if jt < 6:
                nc.scalar.activation(
                    qT[:, jt, sl], pss[tb][:],
                    mybir.ActivationFunctionType.Identity,
                    bias=bias_qk[:, jt : jt + 1],
                )
            else:
                j = jt - 6
                nc.vector.tensor_scalar_add(
                    KTp[0:64, 2 * j, sl], pss[tb][0:64, :],
                    bias_qk[0:64, jt : jt + 1],
                )
                nc.vector.tensor_scalar_add(
                    KTp[64:128, 2 * j + 1, sl], pss[tb][64:128, :],
                    bias_qk[64:128, jt : jt + 1],
                )
    for tt in range(TT):
        pss = [ps512.tile([P, 512], F32, tag="ps512", name=f"vps{i}")[:, :384] for i in range(2)]
        for ct in range(CT):
            for jb in range(2):
                nc.tensor.matmul(
                    pss[jb][:],
                    xT[:, ct, tt * P : (tt + 1) * P],
                    WT[:, ct, 2 * C + jb * 384 : 2 * C + (jb + 1) * 384],
                    start=(ct == 0),
                    stop=(ct == CT - 1),
                )
        for jb in range(2):
            sl = slice(jb * 384, (jb + 1) * 384)
            nc.vector.tensor_add(
                V[:, tt, 6 * jb : 6 * jb + 6, 0:64],
                pss[jb][:].rearrange("p (h d) -> p h d", d=64),
                Bv[:, sl].rearrange("p (h d) -> p h d", d=64),
            )

    # ---- phase 2: per-head attention
    for h in range(H):
        hb = 64 * (h % 2)       # partition base of this head's rows
        vb = hb                 # psum col base for PV output
        db = 64 - hb            # psum col base for denominator
        qj = h // 2

        PT = ptp.tile([P, KT, T], BF16, tag="PT")
        for kt in range(KT):
            qs = kt * P
            for (q0, w) in qk_chunks(kt):
                sps = ps512.tile([P, 512], F32, tag="ps512")
                nc.tensor.matmul(
                    sps[:, :w],
                    KTp[:, h, kt * P : (kt + 1) * P],
                    qT[:, qj, q0 : q0 + w],
                    start=True,
                    stop=True,
                )
                nc.scalar.activation(
                    PT[:, kt, q0 : q0 + w],
                    sps[:, :w],
                    mybir.ActivationFunctionType.Exp,
                    scale=0.125,
                )
            nc.gpsimd.affine_select(
                out=PT[:, kt, qs : qs + P],
                in_=PT[:, kt, qs : qs + P],
                compare_op=mybir.AluOpType.is_ge,
                fill=0.0,
                base=0,
                # keep where (-k + q') >= 0, else fill 0
                pattern=[[1, P]],
                channel_multiplier=-1,
            )

        for b in range(2):
            yD = ps512.tile([P, 512], F32, tag="ps512")
            n_kt = 4 if b == 0 else 8
            for kt in range(n_kt):
                for (off, w) in pv_chunks(kt, b):
                    first = kt == 0
                    last = kt == n_kt - 1
                    rhs = PT[:, kt, b * 512 + off : b * 512 + off + w]
                    nc.tensor.matmul(
                        yD[0:65, off : off + w],
                        V[:, kt, h, :],
                        rhs,
                        start=first,
                        stop=last,
                    )
            Dr = work.tile([P, 512], F32, tag="Dr")
            nc.vector.reciprocal(Dr[64:65, :], yD[64:65, :])
            Drb = work.tile([P, 512], BF16, tag="Drb")
            nc.vector.tensor_copy(Drb[64:65, :], Dr[64:65, :])
            rps = ps512.tile([P, 512], F32, tag="ps512")
            nc.tensor.matmul(
                rps[0:64, :],
                ones_bf[64:65, 0:64],
                Drb[64:65, :],
                start=True,
                stop=True,
                tile_position=(64, 0),
            )
            Rh = work.tile([P, 512], F32, tag="Rh")
            nc.scalar.activation(
                Rh[0:64, :],
                rps[0:64, :],
                mybir.ActivationFunctionType.Copy,
            )
            if hb == 0:
                nc.vector.tensor_mul(
                    yT[0:64, h // 2, b * 512 : (b + 1) * 512],
                    yD[0:64, :],
                    Rh[0:64, :],
                )
            else:
                yTt = work.tile([P, 512], BF16, tag="yTt")
                nc.vector.tensor_mul(yTt[0:64, :], yD[0:64, :], Rh[0:64, :])
                nc.sync.dma_start(
                    yT[64:128, h // 2, b * 512 : (b + 1) * 512], yTt[0:64, :]
                )

    # ---- phase 3: out = y @ Wp^T + b_p
    for tt in range(TT):
        osb = work.tile([P, C], F32, tag="osb")
        pss = [ps512.tile([P, 512], F32, tag="ps512", name=f"vps{i}")[:, :384] for i in range(2)]
        for ct in range(CT):
            for jb in range(2):
                nc.tensor.matmul(
                    pss[jb][:],
                    yT[:, ct, tt * P : (tt + 1) * P],
                    WpT[:, ct, jb * 384 : (jb + 1) * 384],
                    start=(ct == 0),
                    stop=(ct == CT - 1),
                )
        for jb in range(2):
            sl = slice(jb * 384, (jb + 1) * 384)
            nc.vector.tensor_add(osb[:, sl], pss[jb][:], Bp[:, sl])
        nc.sync.dma_start(
            y_d.rearrange("(tt p) c -> tt p c", p=P)[tt], osb[:]
        )


def build_program(loop=1):
    nc = bacc.Bacc("TRN2", target_bir_lowering=False, debug=False)
    x_d = nc.dram_tensor("x", [T, C], F32, kind="ExternalInput").ap()
    wa_d = nc.dram_tensor("W_attn", [3 * C, C], F32, kind="ExternalInput").ap()
    ba_d = nc.dram_tensor("b_attn", [3 * C], F32, kind="ExternalInput").ap()
    wp_d = nc.dram_tensor("W_proj", [C, C], F32, kind="ExternalInput").ap()
    bp_d = nc.dram_tensor("b_proj", [C], F32, kind="ExternalInput").ap()
    y_d = nc.dram_tensor("y", [T, C], F32, kind="ExternalOutput").ap()

    with tile.TileContext(nc) as tc, ExitStack() as ctx:
        const = ctx.enter_context(tc.tile_pool(name="const", bufs=1))
        persist = ctx.enter_context(tc.tile_pool(name="persist", bufs=1))
        nat = ctx.enter_context(tc.tile_pool(name="nat", bufs=6))
        work = ctx.enter_context(tc.tile_pool(name="work", bufs=2))
        ptp = ctx.enter_context(tc.tile_pool(name="ptp", bufs=3))
        dram = ctx.enter_context(tc.tile_pool(name="dram", bufs=1, space="DRAM"))
        ps512 = ctx.enter_context(tc.tile_pool(name="ps512", bufs=8, space="PSUM"))
        ps384 = ps512
        pools = (const, persist, nat, work, ptp, ps512, ps384)

        cst = emit_consts(nc, tc, const, ba_d, bp_d)
        # zero-fill KTp's complementary halves + V ones columns once
        KTp0 = persist.tile([P, 12, T], BF16, tag="KTp")
        nc.gpsimd.memset(KTp0[:], 0.0)
        V0 = persist.tile([P, TT, 12, 65], BF16, tag="Vaug")
        nc.gpsimd.memset(V0[:, :, :, 64:65], 1.0)
        loop_cm = tc.For_i(0, loop, 1) if loop > 1 else contextlib.nullcontext()
        with loop_cm:
            emit_body(nc, tc, pools, cst, dram, x_d, wa_d, wp_d, y_d)

    nc.compile()
    return nc


_CACHED_NC = None


def kernel(x, W_attn, b_attn, W_proj, b_proj):
    from concourse.bass_utils import run_bass_kernel_spmd

    global _CACHED_NC
    if _CACHED_NC is None:
        _CACHED_NC = build_program(loop=1)
    nc = _CACHED_NC

    B = x.shape[0]
    assert B == N_CORES
    in_maps = [
        {
            "x": np.ascontiguousarray(x[b], dtype=np.float32),
            "W_attn": np.asarray(W_attn, dtype=np.float32),
            "b_attn": np.asarray(b_attn, dtype=np.float32),
            "W_proj": np.asarray(W_proj, dtype=np.float32),
            "b_proj": np.asarray(b_proj, dtype=np.float32),
        }
        for b in range(B)
    ]
    res = run_bass_kernel_spmd(nc, in_maps, list(range(N_CORES)))
    return np.stack([res.results[b]["y"] for b in range(B)], axis=0)
